# revision 24
# baseline (speedup 1.0000x reference)
"""Multi-head attention forward, distributed over 8 TRN2 NeuronCores.

Problem: x[2,2048,1024] -> QKV proj (16 heads x 64) -> softmax attention
-> output proj + bias -> [2,2048,1024], f32 I/O, bf16 tensor-engine compute.

Sharding: rows = flattened (batch, seq) = 4096 rows; core c owns rows
[c*512, (c+1)*512) -- cores 0-3 hold batch 0, cores 4-7 batch 1. Each core
projects Q/K/V for its own rows, all-gathers K^T and V (bf16) within its
4-core batch group in 4 key chunks, then computes attention for all 16
heads restricted to its 512 query rows, and the output projection.

Pipeline design (v3):
- A tiny dummy AllGather is emitted first so the NRT kernel-entry CC
  barrier (~20-40us rendezvous) runs while inputs are still streaming in,
  instead of serializing ahead of the first real gather.
- K^T is projected full-width, staged per 128-key chunk into a [256,1024]
  bounce (2KB rows; the v2 [*,128] layout produced 256B DMA descriptors
  that ran at ~6GB/s and stalled the whole projection pipeline).
- Each chunk's AllGather fires as soon as its K+V staging lands; attention
  runs chunk-by-chunk as gathers land. The ACT-engine exp stream (~147us
  for 16.8M score elements at 1 elem/lane/cycle) is the pacing resource;
  everything else hides behind it.
- Scores contract over K=64 via PE row-tiling: head 2p lives in partitions
  0-63 of the K^T/Q^T pair tiles, head 2p+1 in 64-127, and the two matmuls
  run concurrently in different PE row-groups (no zero-padded operands).
- V is augmented with a ones column so the attention matmul itself yields
  the softmax denominator in row 64 (exp has no max subtraction; scores
  are ~N(0,1) after the 1/sqrt(64) scale folded into the ACT scale).
- The V bounce->v_aug interleave copy runs on GpSimd so the Vector queue
  (PSUM evacuations, accumulator adds) never blocks chunk unpacking.
"""

import ml_dtypes
import numpy as np

import concourse.bass as bass
import concourse.mybir as mybir
import concourse.tile as tile
from concourse import bacc
from concourse.bass_utils import run_bass_kernel_spmd

BF = mybir.dt.bfloat16
F32 = mybir.dt.float32
F8 = mybir.dt.float8e4
P = 128

N_CORES = 8
GROUP = 4   # cores per batch group (one AllGather group)
NCH = 4     # AllGather chunks (pipelined)


class Cfg:
    def __init__(self, rpc, d, n_heads, head_dim):
        self.RPC = rpc            # query rows per core
        self.D = d                # model dim
        self.H = n_heads
        self.HD = head_dim
        assert n_heads * head_dim == d
        self.NT_D = d // P        # dim tiles (= head pairs)
        self.NT_R = rpc // P      # row tiles
        self.KEYS = rpc * GROUP   # keys per batch group
        self.NT_K = self.KEYS // P
        self.KPC = rpc // NCH     # local keys per chunk
        assert self.KPC == P      # one key tile per (rank, chunk)
        # bounce rows per rank chunk: K^T only (V is computed locally from
        # the host-replicated group x), [KPC, D] with row=partition of the
        # wide K^T tile, col=m*P+key (see ktw)
        self.CH_ROWS = self.KPC


FULL = Cfg(rpc=512, d=1024, n_heads=16, head_dim=64)


def _body(tc, nc, cfg, xT_in, xg_in, wq_in, wk_in, wv_in, wo_in, bo_in, out_ext):
    c = cfg
    AF = mybir.ActivationFunctionType
    rg = [list(range(GROUP)), list(range(GROUP, 2 * GROUP))]
    HD1 = c.HD + 1
    from contextlib import ExitStack

    stack = ExitStack()
    dram = stack.enter_context(tc.tile_pool(name="dram", bufs=1, space="DRAM"))
    const = stack.enter_context(tc.tile_pool(name="const", bufs=1))
    persist = stack.enter_context(tc.tile_pool(name="persist", bufs=1))

    kv_in = [dram.tile([c.CH_ROWS, c.D], BF, name=f"kv_in{h}") for h in range(NCH)]
    kv_g = [
        dram.tile([GROUP * c.CH_ROWS, c.D], BF, name=f"kv_g{h}") for h in range(NCH)
    ]
    ones_row = const.tile([1, P], BF, tag="ones_row", name="ones_row")
    nc.vector.memset(ones_row[:], 1.0)
    bo_sb = const.tile([1, c.D], BF, tag="bo", name="bo_sb")
    nc.sync.dma_start(bo_sb[:], bo_in[:, :])
    # Pre-warm the ACT exp table so the ~2.7us table load is off the
    # attention critical path.
    warm_act = const.tile([1, P], BF, tag="warm_act", name="warm_act")
    nc.scalar.activation(warm_act[:], ones_row[:], AF.Exp)

    def ptiles(shape, dt_, pfx, n, pool=None):
        pool = pool or persist
        return [pool.tile(shape, dt_, tag=f"{pfx}{t}", name=f"{pfx}{t}") for t in range(n)]

    xT = ptiles([P, c.RPC], BF, "xT", c.NT_D)
    xg = ptiles([P, c.KEYS], BF, "xg", c.NT_D)          # group x^T (all keys)
    qT = ptiles([P, c.RPC], BF, "qT", c.NT_D)
    attT = ptiles([P, c.RPC], BF, "attT", c.NT_D)
    kt = ptiles([P, c.KEYS], BF, "kt", c.NT_D)          # K^T pair tiles, all keys
    v_aug = ptiles([P, c.H * HD1], BF, "va", c.NT_K)    # V + ones col, per key tile
    acc_eo = ptiles([HD1, 2 * c.RPC], BF, "acc", c.NT_D)

    def emit_ag(h):
        nc.gpsimd.collective_compute(
            "AllGather",
            mybir.AluOpType.bypass,
            replica_groups=rg,
            ins=[kv_in[h][:].opt()],
            outs=[kv_g[h][:].opt()],
        )

    with (
        tc.tile_pool(name="stage", bufs=1) as stage,
        tc.tile_pool(name="wpool", bufs=1) as wpool,
        tc.tile_pool(name="proj_psum", bufs=3, space="PSUM") as proj_psum,
    ):
        wq_sb = ptiles([P, c.D], BF, "wq", c.NT_D, pool=wpool)
        wk_sb = ptiles([P, c.D], BF, "wk", c.NT_D, pool=wpool)
        # wide K^T staging tile: ktw[p, h*D + m*P + k] = K^T[m*P+p, h*KPC+k]
        ktw = stage.tile([P, NCH * c.D], BF, tag="ktw", name="ktw")

        # ---- phase 0: load x^T and weights (already bf16, pre-transposed).
        for t in range(c.NT_D):
            nc.sync.dma_start(xT[t][:], xT_in[t * P : (t + 1) * P, :])
            nc.sync.dma_start(wk_sb[t][:], wk_in[t * P : (t + 1) * P, :])
        for t in range(c.NT_D):
            nc.sync.dma_start(wq_sb[t][:], wq_in[t * P : (t + 1) * P, :])
        for h in range(NCH):
            for t in range(c.NT_D):
                nc.sync.dma_start(
                    xg[t][:, h * GROUP * P : (h + 1) * GROUP * P],
                    xg_in[t * P : (t + 1) * P, h * GROUP * P : (h + 1) * GROUP * P],
                )

        # ---- phase 1a: full-width K^T projection, evac sliced per chunk ----
        for m in range(c.NT_D):
            ps = proj_psum.tile([P, c.RPC], F32, tag="proj", name="kproj_ps")
            for k in range(c.NT_D):
                nc.tensor.matmul(
                    ps[:],
                    wk_sb[k][:, m * P : (m + 1) * P],
                    xT[k][:],
                    start=(k == 0),
                    stop=(k == c.NT_D - 1),
                )
            # one strided copy scatters the 4 chunk slices of this m
            nc.vector.tensor_copy(
                ktw[:].rearrange("p (h mk) -> p h mk", h=NCH)[
                    :, :, m * P : (m + 1) * P
                ],
                ps[:].rearrange("p (h k) -> p h k", h=NCH),
            )

        # ---- phase 1b: stage each K chunk -> AllGather (K only) ----
        for h in range(NCH):
            nc.sync.dma_start(
                kv_in[h][0 : c.KPC, :], ktw[:, h * c.D : (h + 1) * c.D]
            )
            emit_ag(h)
            if h == 0:
                # Q^T projection overlaps the first AllGather's flight time
                for m in range(c.NT_D):
                    ps = proj_psum.tile([P, c.RPC], F32, tag="proj", name="qproj_ps")
                    for k in range(c.NT_D):
                        nc.tensor.matmul(
                            ps[:],
                            wq_sb[k][:, m * P : (m + 1) * P],
                            xT[k][:],
                            start=(k == 0),
                            stop=(k == c.NT_D - 1),
                        )
                    nc.vector.tensor_copy(qT[m][:], ps[:])

    # ---- phases 2+3: per chunk: unpack gathered K/V, then attention ----
    # key-tile order is chunk-major: tile j = h*GROUP + r covers gathered
    # chunk h of rank r; kt cols [j*P, (j+1)*P) and v_aug[j] use it.
    with (
        tc.tile_pool(name="wopool", bufs=1) as wopool,
        tc.tile_pool(name="pT", bufs=4) as pT_pool,
        tc.tile_pool(name="small", bufs=4) as small,
        tc.tile_pool(name="sc_psum", bufs=2, space="PSUM") as sc_psum,
        tc.tile_pool(name="att_psum", bufs=1, space="PSUM") as att_psum,
        tc.tile_pool(name="vproj_psum", bufs=2, space="PSUM") as vproj_psum,
    ):
        wo_sb = ptiles([P, c.D], BF, "wo", c.NT_D, pool=wopool)
        wv_sb = ptiles([P, c.D], BF, "wv", c.NT_D, pool=wopool)
        for t in range(c.NT_D):
            nc.sync.dma_start(wv_sb[t][:], wv_in[t * P : (t + 1) * P, :])

        def v_half(j, n):
            """One 512-col half of V-tile j's projection (see v_tile)."""
            gb = (j % GROUP) * c.RPC + (j // GROUP) * c.KPC
            ps = vproj_psum.tile([P, c.RPC], F32, tag="vproj", name="vproj_ps")
            for k in range(c.NT_D):
                nc.tensor.matmul(
                    ps[:],
                    xg[k][:, gb : gb + P],
                    wv_sb[k][:, n * c.RPC : (n + 1) * c.RPC],
                    start=(k == 0),
                    stop=(k == c.NT_D - 1),
                )
            nc.vector.tensor_copy(
                v_aug[j][:, n * 8 * HD1 : (n + 1) * 8 * HD1].rearrange(
                    "p (x e) -> p x e", e=HD1
                )[:, :, 0 : c.HD],
                ps[:].rearrange("p (x e) -> p x e", e=c.HD),
            )
            if n == 1:
                ones_col = v_aug[j][:].rearrange("p (x e) -> p x e", e=HD1)[
                    :, :, c.HD : HD1
                ]
                nc.gpsimd.memset(ones_col, 1.0)

        def v_tile(j):
            """V projection for key tile j from the replicated group x,
            evacuated straight into the ones-augmented [head, 65] layout
            (one strided copy per half).  Key tile j follows the gathered
            chunk-major order: j = chunk*GROUP + rank covers group rows
            rank*RPC + chunk*KPC (xg columns are in natural group order)."""
            gb = (j % GROUP) * c.RPC + (j // GROUP) * c.KPC
            for n in range(2):
                ps = vproj_psum.tile([P, c.RPC], F32, tag="vproj", name="vproj_ps")
                for k in range(c.NT_D):
                    nc.tensor.matmul(
                        ps[:],
                        xg[k][:, gb : gb + P],
                        wv_sb[k][:, n * c.RPC : (n + 1) * c.RPC],
                        start=(k == 0),
                        stop=(k == c.NT_D - 1),
                    )
                nc.vector.tensor_copy(
                    v_aug[j][:, n * 8 * HD1 : (n + 1) * 8 * HD1].rearrange(
                        "p (x e) -> p x e", e=HD1
                    )[:, :, 0 : c.HD],
                    ps[:].rearrange("p (x e) -> p x e", e=c.HD),
                )
            ones_col = v_aug[j][:].rearrange("p (x e) -> p x e", e=HD1)[
                :, :, c.HD : HD1
            ]
            nc.gpsimd.memset(ones_col, 1.0)

        # chunk-0 V tiles up front (TensorE work, overlaps the first gather)
        for j in range(GROUP):
            v_tile(j)

        for h in range(NCH):
            if h == 1:
                # Wo loads are off the critical path; emit after chunk 0's
                # unpack so they don't contend with the staging window
                for t in range(c.NT_D):
                    nc.sync.dma_start(wo_sb[t][:], wo_in[t * P : (t + 1) * P, :])

            # unpack K: per m, one DMA grabs all 4 ranks' key blocks
            for m in range(c.NT_D):
                nc.sync.dma_start(
                    kt[m][:, h * GROUP * P : (h + 1) * GROUP * P].rearrange(
                        "p (r k) -> p r k", r=GROUP
                    ),
                    kv_g[h][:, m * P : (m + 1) * P].rearrange(
                        "(r q) k -> q r k", r=GROUP
                    )[0:P, :, :],
                )
            for p in range(c.NT_D):
                he, ho = 2 * p, 2 * p + 1
                att_eo = att_psum.tile([HD1, 2 * c.RPC], F32, tag="att_eo", name="att_eo")
                for i in range(GROUP):
                    j = h * GROUP + i
                    col = j * P
                    sc = sc_psum.tile([P, 2 * c.RPC], F32, tag="scores", name="sc_ps")
                    # even/odd heads of the pair run concurrently in PE
                    # row-groups 0 and 2 (K=64 row tiling)
                    nc.tensor.matmul(
                        sc[:, 0 : c.RPC],
                        kt[p][0 : c.HD, col : col + P],
                        qT[p][0 : c.HD, :],
                        start=True,
                        stop=True,
                    )
                    nc.tensor.matmul(
                        sc[:, c.RPC : 2 * c.RPC],
                        kt[p][c.HD : P, col : col + P],
                        qT[p][c.HD : P, :],
                        start=True,
                        stop=True,
                    )
                    pT = pT_pool.tile([P, 2 * c.RPC], BF, tag="pT", name="pT")
                    nc.scalar.activation(
                        pT[:], sc[:], AF.Exp, scale=1.0 / float(np.sqrt(c.HD))
                    )
                    nc.tensor.matmul(
                        att_eo[:, 0 : c.RPC],
                        v_aug[j][:, he * HD1 : (he + 1) * HD1],
                        pT[:, 0 : c.RPC],
                        start=(i == 0),
                        stop=(i == GROUP - 1),
                    )
                    nc.tensor.matmul(
                        att_eo[:, c.RPC : 2 * c.RPC],
                        v_aug[j][:, ho * HD1 : (ho + 1) * HD1],
                        pT[:, c.RPC : 2 * c.RPC],
                        start=(i == 0),
                        stop=(i == GROUP - 1),
                    )
                if h == 0:
                    nc.vector.tensor_copy(acc_eo[p][:], att_eo[:])
                else:
                    nc.vector.tensor_add(acc_eo[p][:], att_eo[:], acc_eo[p][:])

                if h + 1 < NCH:
                    # next chunk's V projections, one half-tile chain per
                    # pair slot so TensorE work per slot (~2us) stays under
                    # the exp stream's ~4.6us pair cadence
                    v_half((h + 1) * GROUP + p // 2, p % 2)

                if h == NCH - 1:
                    # normalization: denominators live in row HD of the accs.
                    # reciprocal_approx_fast is a custom DVE program -- feed
                    # it a partition-0-based tile, not a row-64 slice.
                    den = small.tile([1, 2 * c.RPC], F32, tag="den", name="den", bufs=1)
                    nc.vector.tensor_copy(den[:], acc_eo[p][c.HD : HD1, :])
                    rcp = small.tile([1, 2 * c.RPC], F32, tag="rcp", name="rcp", bufs=1)
                    nc.vector.reciprocal_approx_fast(rcp[:], den[:])
                    rcpb = small.tile([c.HD, 2 * c.RPC], F32, tag="rcpb", name="rcpb", bufs=1)
                    nc.gpsimd.partition_broadcast(rcpb[:], rcp[:])
                    nc.vector.tensor_mul(
                        attT[p][0 : c.HD, :], acc_eo[p][0 : c.HD, 0 : c.RPC],
                        rcpb[:, 0 : c.RPC],
                    )
                    nc.vector.tensor_mul(
                        attT[p][c.HD : P, :], acc_eo[p][0 : c.HD, c.RPC : 2 * c.RPC],
                        rcpb[:, c.RPC : 2 * c.RPC],
                    )

        # ---- phase 4: output projection + bias ----
        for rt in range(c.NT_R):
            out_sb = small.tile([P, c.D], F32, tag="outsb", name="outsb", bufs=2)
            for n in range(2):
                po = att_psum.tile([P, c.RPC], F32, tag="att_eo", name="out_ps")
                for k in range(c.NT_D):
                    nc.tensor.matmul(
                        po[:],
                        attT[k][:, rt * P : (rt + 1) * P],
                        wo_sb[k][:, n * c.RPC : (n + 1) * c.RPC],
                        start=(k == 0),
                        stop=False,
                    )
                nc.tensor.matmul(
                    po[:],
                    ones_row[:],
                    bo_sb[:, n * c.RPC : (n + 1) * c.RPC],
                    start=False,
                    stop=True,
                )
                nc.vector.tensor_copy(out_sb[:, n * c.RPC : (n + 1) * c.RPC], po[:])
            nc.sync.dma_start(out_ext[rt * P : (rt + 1) * P, :], out_sb[:])

    stack.close()


def build_nc(cfg):
    nc = bacc.Bacc(
        "TRN2", target_bir_lowering=False, debug=False, num_devices=N_CORES
    )
    c = cfg
    xT_in = nc.dram_tensor("xT", [c.D, c.RPC], BF, kind="ExternalInput")
    xg_in = nc.dram_tensor("xg", [c.D, c.KEYS], BF, kind="ExternalInput")
    wq_in = nc.dram_tensor("Wq", [c.D, c.D], BF, kind="ExternalInput")
    wk_in = nc.dram_tensor("Wk", [c.D, c.D], BF, kind="ExternalInput")
    wv_in = nc.dram_tensor("Wv", [c.D, c.D], BF, kind="ExternalInput")
    wo_in = nc.dram_tensor("Wo", [c.D, c.D], BF, kind="ExternalInput")
    bo_in = nc.dram_tensor("bo", [1, c.D], BF, kind="ExternalInput")
    out_ext = nc.dram_tensor("out", [c.RPC, c.D], F32, kind="ExternalOutput")

    with tile.TileContext(nc) as tc:
        _body(
            tc, nc, cfg,
            xT_in.ap(), xg_in.ap(), wq_in.ap(), wk_in.ap(), wv_in.ap(),
            wo_in.ap(), bo_in.ap(), out_ext.ap(),
        )
    nc.compile()
    return nc


_cached_nc = None


def _bf16(a):
    return np.ascontiguousarray(np.asarray(a, dtype=np.float32)).astype(
        ml_dtypes.bfloat16
    )


def prep_in_maps(c, x, Wq, Wk, Wv, Wo, bo):
    xf = np.ascontiguousarray(np.asarray(x, dtype=np.float32)).reshape(-1, c.D)
    wq, wk, wv, wo = _bf16(Wq), _bf16(Wk), _bf16(Wv), _bf16(Wo)
    bob = _bf16(bo).reshape(1, c.D)
    xgs = [
        np.ascontiguousarray(
            xf[g * c.KEYS : (g + 1) * c.KEYS].T.astype(ml_dtypes.bfloat16)
        )
        for g in range(N_CORES // GROUP)
    ]
    return [
        {
            "xT": np.ascontiguousarray(
                xf[cid * c.RPC : (cid + 1) * c.RPC].T.astype(ml_dtypes.bfloat16)
            ),
            "xg": xgs[cid // GROUP],
            "Wq": wq, "Wk": wk, "Wv": wv, "Wo": wo, "bo": bob,
        }
        for cid in range(N_CORES)
    ]


def kernel(x, Wq, Wk, Wv, Wo, bo):
    global _cached_nc
    c = FULL
    if _cached_nc is None:
        _cached_nc = build_nc(c)
    nc = _cached_nc

    in_maps = prep_in_maps(c, x, Wq, Wk, Wv, Wo, bo)
    res = run_bass_kernel_spmd(nc, in_maps, list(range(N_CORES)))
    out = np.concatenate([res.results[cid]["out"] for cid in range(N_CORES)], axis=0)
    return out.reshape(np.asarray(x).shape).astype(np.float32)


# revision 25
# speedup vs baseline: 1.0120x; 1.0120x over previous
"""Multi-head attention forward, distributed over 8 TRN2 NeuronCores.

Problem: x[2,2048,1024] -> QKV proj (16 heads x 64) -> softmax attention
-> output proj + bias -> [2,2048,1024], f32 I/O, bf16 tensor-engine compute.

Sharding: rows = flattened (batch, seq) = 4096 rows; core c owns rows
[c*512, (c+1)*512) -- cores 0-3 hold batch 0, cores 4-7 batch 1. Each core
projects Q/K/V for its own rows, all-gathers K^T and V (bf16) within its
4-core batch group in 4 key chunks, then computes attention for all 16
heads restricted to its 512 query rows, and the output projection.

Pipeline design (v3):
- A tiny dummy AllGather is emitted first so the NRT kernel-entry CC
  barrier (~20-40us rendezvous) runs while inputs are still streaming in,
  instead of serializing ahead of the first real gather.
- K^T is projected full-width, staged per 128-key chunk into a [256,1024]
  bounce (2KB rows; the v2 [*,128] layout produced 256B DMA descriptors
  that ran at ~6GB/s and stalled the whole projection pipeline).
- Each chunk's AllGather fires as soon as its K+V staging lands; attention
  runs chunk-by-chunk as gathers land. The ACT-engine exp stream (~147us
  for 16.8M score elements at 1 elem/lane/cycle) is the pacing resource;
  everything else hides behind it.
- Scores contract over K=64 via PE row-tiling: head 2p lives in partitions
  0-63 of the K^T/Q^T pair tiles, head 2p+1 in 64-127, and the two matmuls
  run concurrently in different PE row-groups (no zero-padded operands).
- V is augmented with a ones column so the attention matmul itself yields
  the softmax denominator in row 64 (exp has no max subtraction; scores
  are ~N(0,1) after the 1/sqrt(64) scale folded into the ACT scale).
- The V bounce->v_aug interleave copy runs on GpSimd so the Vector queue
  (PSUM evacuations, accumulator adds) never blocks chunk unpacking.
"""

import ml_dtypes
import numpy as np

import concourse.bass as bass
import concourse.mybir as mybir
import concourse.tile as tile
from concourse import bacc
from concourse.bass_utils import run_bass_kernel_spmd

BF = mybir.dt.bfloat16
F32 = mybir.dt.float32
F8 = mybir.dt.float8e4
P = 128

N_CORES = 8
GROUP = 4   # cores per batch group (one AllGather group)
NCH = 4     # AllGather chunks (pipelined)


class Cfg:
    def __init__(self, rpc, d, n_heads, head_dim):
        self.RPC = rpc            # query rows per core
        self.D = d                # model dim
        self.H = n_heads
        self.HD = head_dim
        assert n_heads * head_dim == d
        self.NT_D = d // P        # dim tiles (= head pairs)
        self.NT_R = rpc // P      # row tiles
        self.KEYS = rpc * GROUP   # keys per batch group
        self.NT_K = self.KEYS // P
        self.KPC = rpc // NCH     # local keys per chunk
        assert self.KPC == P      # one key tile per (rank, chunk)
        # bounce rows per rank chunk: K^T only (V is computed locally from
        # the host-replicated group x), [KPC, D] with row=partition of the
        # wide K^T tile, col=m*P+key (see ktw)
        self.CH_ROWS = self.KPC


FULL = Cfg(rpc=512, d=1024, n_heads=16, head_dim=64)


def _body(tc, nc, cfg, xT_in, xg_in, wq_in, wk_in, wv_in, wo_in, bo_in, out_ext):
    c = cfg
    AF = mybir.ActivationFunctionType
    rg = [list(range(GROUP)), list(range(GROUP, 2 * GROUP))]
    HD1 = c.HD + 1
    from contextlib import ExitStack

    stack = ExitStack()
    dram = stack.enter_context(tc.tile_pool(name="dram", bufs=1, space="DRAM"))
    const = stack.enter_context(tc.tile_pool(name="const", bufs=1))
    persist = stack.enter_context(tc.tile_pool(name="persist", bufs=1))

    kv_in = [dram.tile([c.CH_ROWS, c.D], BF, name=f"kv_in{h}") for h in range(NCH)]
    kv_g = [
        dram.tile([GROUP * c.CH_ROWS, c.D], BF, name=f"kv_g{h}") for h in range(NCH)
    ]
    ones_row = const.tile([1, P], BF, tag="ones_row", name="ones_row")
    nc.vector.memset(ones_row[:], 1.0)
    bo_sb = const.tile([1, c.D], BF, tag="bo", name="bo_sb")
    nc.sync.dma_start(bo_sb[:], bo_in[:, :])
    # Pre-warm the ACT exp table so the ~2.7us table load is off the
    # attention critical path.
    warm_act = const.tile([1, P], BF, tag="warm_act", name="warm_act")
    nc.scalar.activation(warm_act[:], ones_row[:], AF.Exp)

    def ptiles(shape, dt_, pfx, n, pool=None):
        pool = pool or persist
        return [pool.tile(shape, dt_, tag=f"{pfx}{t}", name=f"{pfx}{t}") for t in range(n)]

    xT = ptiles([P, c.RPC], BF, "xT", c.NT_D)
    xg = ptiles([P, c.KEYS], BF, "xg", c.NT_D)          # group x^T (all keys)
    qT = ptiles([P, c.RPC], BF, "qT", c.NT_D)
    attT = ptiles([P, c.RPC], BF, "attT", c.NT_D)
    kt = ptiles([P, c.KEYS], BF, "kt", c.NT_D)          # K^T pair tiles, all keys
    v_aug = ptiles([P, c.H * HD1], BF, "va", c.NT_K)    # V + ones col, per key tile
    acc_eo = ptiles([HD1, 2 * c.RPC], BF, "acc", c.NT_D)

    def emit_ag(h):
        nc.gpsimd.collective_compute(
            "AllGather",
            mybir.AluOpType.bypass,
            replica_groups=rg,
            ins=[kv_in[h][:].opt()],
            outs=[kv_g[h][:].opt()],
        )

    with (
        tc.tile_pool(name="stage", bufs=1) as stage,
        tc.tile_pool(name="wpool", bufs=1) as wpool,
        tc.tile_pool(name="proj_psum", bufs=3, space="PSUM") as proj_psum,
    ):
        wq_sb = ptiles([P, c.D], BF, "wq", c.NT_D, pool=wpool)
        wk_sb = ptiles([P, c.D], BF, "wk", c.NT_D, pool=wpool)
        # wide K^T staging tile: ktw[p, h*D + m*P + k] = K^T[m*P+p, h*KPC+k]
        ktw = stage.tile([P, NCH * c.D], BF, tag="ktw", name="ktw")

        # ---- phase 0: load x^T and weights (already bf16, pre-transposed).
        for t in range(c.NT_D):
            nc.sync.dma_start(xT[t][:], xT_in[t * P : (t + 1) * P, :])
            nc.sync.dma_start(wk_sb[t][:], wk_in[t * P : (t + 1) * P, :])
        for t in range(c.NT_D):
            nc.sync.dma_start(wq_sb[t][:], wq_in[t * P : (t + 1) * P, :])
        for h in range(NCH):
            for t in range(c.NT_D):
                nc.sync.dma_start(
                    xg[t][:, h * GROUP * P : (h + 1) * GROUP * P],
                    xg_in[t * P : (t + 1) * P, h * GROUP * P : (h + 1) * GROUP * P],
                )

        # ---- phase 1a: full-width K^T projection, evac sliced per chunk ----
        for m in range(c.NT_D):
            ps = proj_psum.tile([P, c.RPC], F32, tag="proj", name="kproj_ps")
            for k in range(c.NT_D):
                nc.tensor.matmul(
                    ps[:],
                    wk_sb[k][:, m * P : (m + 1) * P],
                    xT[k][:],
                    start=(k == 0),
                    stop=(k == c.NT_D - 1),
                )
            # one strided copy scatters the 4 chunk slices of this m
            nc.vector.tensor_copy(
                ktw[:].rearrange("p (h mk) -> p h mk", h=NCH)[
                    :, :, m * P : (m + 1) * P
                ],
                ps[:].rearrange("p (h k) -> p h k", h=NCH),
            )

        # ---- phase 1b: stage each K chunk -> AllGather (K only) ----
        for h in range(NCH):
            nc.sync.dma_start(
                kv_in[h][0 : c.KPC, :], ktw[:, h * c.D : (h + 1) * c.D]
            )
            emit_ag(h)
            if h == 0:
                # Q^T projection overlaps the first AllGather's flight time
                for m in range(c.NT_D):
                    ps = proj_psum.tile([P, c.RPC], F32, tag="proj", name="qproj_ps")
                    for k in range(c.NT_D):
                        nc.tensor.matmul(
                            ps[:],
                            wq_sb[k][:, m * P : (m + 1) * P],
                            xT[k][:],
                            start=(k == 0),
                            stop=(k == c.NT_D - 1),
                        )
                    nc.vector.tensor_copy(qT[m][:], ps[:])

    # ---- phases 2+3: per chunk: unpack gathered K/V, then attention ----
    # key-tile order is chunk-major: tile j = h*GROUP + r covers gathered
    # chunk h of rank r; kt cols [j*P, (j+1)*P) and v_aug[j] use it.
    with (
        tc.tile_pool(name="wopool", bufs=1) as wopool,
        tc.tile_pool(name="pT", bufs=4) as pT_pool,
        tc.tile_pool(name="small", bufs=4) as small,
        tc.tile_pool(name="sc_psum", bufs=2, space="PSUM") as sc_psum,
        tc.tile_pool(name="att_psum", bufs=1, space="PSUM") as att_psum,
        tc.tile_pool(name="vproj_psum", bufs=2, space="PSUM") as vproj_psum,
    ):
        wo_sb = ptiles([P, c.D], BF, "wo", c.NT_D, pool=wopool)
        wv_sb = ptiles([P, c.D], BF, "wv", c.NT_D, pool=wopool)
        for t in range(c.NT_D):
            nc.sync.dma_start(wv_sb[t][:], wv_in[t * P : (t + 1) * P, :])

        def v_tile(j):
            """V projection for key tile j from the replicated group x,
            evacuated straight into the ones-augmented [head, 65] layout
            (one strided copy per half).  Key tile j follows the gathered
            chunk-major order: j = chunk*GROUP + rank covers group rows
            rank*RPC + chunk*KPC (xg columns are in natural group order)."""
            gb = (j % GROUP) * c.RPC + (j // GROUP) * c.KPC
            for n in range(2):
                ps = vproj_psum.tile([P, c.RPC], F32, tag="vproj", name="vproj_ps")
                for k in range(c.NT_D):
                    nc.tensor.matmul(
                        ps[:],
                        xg[k][:, gb : gb + P],
                        wv_sb[k][:, n * c.RPC : (n + 1) * c.RPC],
                        start=(k == 0),
                        stop=(k == c.NT_D - 1),
                    )
                nc.vector.tensor_copy(
                    v_aug[j][:, n * 8 * HD1 : (n + 1) * 8 * HD1].rearrange(
                        "p (x e) -> p x e", e=HD1
                    )[:, :, 0 : c.HD],
                    ps[:].rearrange("p (x e) -> p x e", e=c.HD),
                )
            ones_col = v_aug[j][:].rearrange("p (x e) -> p x e", e=HD1)[
                :, :, c.HD : HD1
            ]
            nc.gpsimd.memset(ones_col, 1.0)

        # chunk-0 V tiles up front (TensorE work, overlaps the first gather)
        for j in range(GROUP):
            v_tile(j)

        for h in range(NCH):
            if h == 1:
                # Wo loads are off the critical path; emit after chunk 0's
                # unpack so they don't contend with the staging window
                for t in range(c.NT_D):
                    nc.sync.dma_start(wo_sb[t][:], wo_in[t * P : (t + 1) * P, :])

            # unpack K: per m, one DMA grabs all 4 ranks' key blocks
            for m in range(c.NT_D):
                nc.sync.dma_start(
                    kt[m][:, h * GROUP * P : (h + 1) * GROUP * P].rearrange(
                        "p (r k) -> p r k", r=GROUP
                    ),
                    kv_g[h][:, m * P : (m + 1) * P].rearrange(
                        "(r q) k -> q r k", r=GROUP
                    )[0:P, :, :],
                )
            for p in range(c.NT_D):
                he, ho = 2 * p, 2 * p + 1
                att_eo = att_psum.tile([HD1, 2 * c.RPC], F32, tag="att_eo", name="att_eo")
                for i in range(GROUP):
                    j = h * GROUP + i
                    col = j * P
                    sc = sc_psum.tile([P, 2 * c.RPC], F32, tag="scores", name="sc_ps")
                    # even/odd heads of the pair run concurrently in PE
                    # row-groups 0 and 2 (K=64 row tiling)
                    nc.tensor.matmul(
                        sc[:, 0 : c.RPC],
                        kt[p][0 : c.HD, col : col + P],
                        qT[p][0 : c.HD, :],
                        start=True,
                        stop=True,
                    )
                    nc.tensor.matmul(
                        sc[:, c.RPC : 2 * c.RPC],
                        kt[p][c.HD : P, col : col + P],
                        qT[p][c.HD : P, :],
                        start=True,
                        stop=True,
                    )
                    pT = pT_pool.tile([P, 2 * c.RPC], BF, tag="pT", name="pT")
                    nc.scalar.activation(
                        pT[:], sc[:], AF.Exp, scale=1.0 / float(np.sqrt(c.HD))
                    )
                    nc.tensor.matmul(
                        att_eo[:, 0 : c.RPC],
                        v_aug[j][:, he * HD1 : (he + 1) * HD1],
                        pT[:, 0 : c.RPC],
                        start=(i == 0),
                        stop=(i == GROUP - 1),
                    )
                    nc.tensor.matmul(
                        att_eo[:, c.RPC : 2 * c.RPC],
                        v_aug[j][:, ho * HD1 : (ho + 1) * HD1],
                        pT[:, c.RPC : 2 * c.RPC],
                        start=(i == 0),
                        stop=(i == GROUP - 1),
                    )
                if h == 0:
                    nc.vector.tensor_copy(acc_eo[p][:], att_eo[:])
                else:
                    nc.vector.tensor_add(acc_eo[p][:], att_eo[:], acc_eo[p][:])

                if h + 1 < NCH and p < GROUP:
                    # next chunk's V tiles, one per pair slot (TensorE has
                    # slack under the exp stream)
                    v_tile((h + 1) * GROUP + p)

                if h == NCH - 1:
                    # normalization: denominators live in row HD of the accs.
                    # reciprocal_approx_fast is a custom DVE program -- feed
                    # it a partition-0-based tile, not a row-64 slice.
                    den = small.tile([1, 2 * c.RPC], F32, tag="den", name="den", bufs=1)
                    nc.vector.tensor_copy(den[:], acc_eo[p][c.HD : HD1, :])
                    rcp = small.tile([1, 2 * c.RPC], F32, tag="rcp", name="rcp", bufs=1)
                    nc.vector.reciprocal_approx_fast(rcp[:], den[:])
                    rcpb = small.tile([c.HD, 2 * c.RPC], F32, tag="rcpb", name="rcpb", bufs=1)
                    nc.gpsimd.partition_broadcast(rcpb[:], rcp[:])
                    nc.vector.tensor_mul(
                        attT[p][0 : c.HD, :], acc_eo[p][0 : c.HD, 0 : c.RPC],
                        rcpb[:, 0 : c.RPC],
                    )
                    nc.vector.tensor_mul(
                        attT[p][c.HD : P, :], acc_eo[p][0 : c.HD, c.RPC : 2 * c.RPC],
                        rcpb[:, c.RPC : 2 * c.RPC],
                    )

        # ---- phase 4: output projection + bias ----
        for rt in range(c.NT_R):
            out_sb = small.tile([P, c.D], F32, tag="outsb", name="outsb", bufs=2)
            for n in range(2):
                po = att_psum.tile([P, c.RPC], F32, tag="att_eo", name="out_ps")
                for k in range(c.NT_D):
                    nc.tensor.matmul(
                        po[:],
                        attT[k][:, rt * P : (rt + 1) * P],
                        wo_sb[k][:, n * c.RPC : (n + 1) * c.RPC],
                        start=(k == 0),
                        stop=False,
                    )
                nc.tensor.matmul(
                    po[:],
                    ones_row[:],
                    bo_sb[:, n * c.RPC : (n + 1) * c.RPC],
                    start=False,
                    stop=True,
                )
                nc.vector.tensor_copy(out_sb[:, n * c.RPC : (n + 1) * c.RPC], po[:])
            nc.sync.dma_start(out_ext[rt * P : (rt + 1) * P, :], out_sb[:])

    stack.close()


def build_nc(cfg):
    nc = bacc.Bacc(
        "TRN2", target_bir_lowering=False, debug=False, num_devices=N_CORES
    )
    c = cfg
    xT_in = nc.dram_tensor("xT", [c.D, c.RPC], BF, kind="ExternalInput")
    xg_in = nc.dram_tensor("xg", [c.D, c.KEYS], BF, kind="ExternalInput")
    wq_in = nc.dram_tensor("Wq", [c.D, c.D], BF, kind="ExternalInput")
    wk_in = nc.dram_tensor("Wk", [c.D, c.D], BF, kind="ExternalInput")
    wv_in = nc.dram_tensor("Wv", [c.D, c.D], BF, kind="ExternalInput")
    wo_in = nc.dram_tensor("Wo", [c.D, c.D], BF, kind="ExternalInput")
    bo_in = nc.dram_tensor("bo", [1, c.D], BF, kind="ExternalInput")
    out_ext = nc.dram_tensor("out", [c.RPC, c.D], F32, kind="ExternalOutput")

    with tile.TileContext(nc) as tc:
        _body(
            tc, nc, cfg,
            xT_in.ap(), xg_in.ap(), wq_in.ap(), wk_in.ap(), wv_in.ap(),
            wo_in.ap(), bo_in.ap(), out_ext.ap(),
        )
    nc.compile()
    return nc


_cached_nc = None


def _bf16(a):
    return np.ascontiguousarray(np.asarray(a, dtype=np.float32)).astype(
        ml_dtypes.bfloat16
    )


def prep_in_maps(c, x, Wq, Wk, Wv, Wo, bo):
    xf = np.ascontiguousarray(np.asarray(x, dtype=np.float32)).reshape(-1, c.D)
    wq, wk, wv, wo = _bf16(Wq), _bf16(Wk), _bf16(Wv), _bf16(Wo)
    bob = _bf16(bo).reshape(1, c.D)
    xgs = [
        np.ascontiguousarray(
            xf[g * c.KEYS : (g + 1) * c.KEYS].T.astype(ml_dtypes.bfloat16)
        )
        for g in range(N_CORES // GROUP)
    ]
    return [
        {
            "xT": np.ascontiguousarray(
                xf[cid * c.RPC : (cid + 1) * c.RPC].T.astype(ml_dtypes.bfloat16)
            ),
            "xg": xgs[cid // GROUP],
            "Wq": wq, "Wk": wk, "Wv": wv, "Wo": wo, "bo": bob,
        }
        for cid in range(N_CORES)
    ]


def kernel(x, Wq, Wk, Wv, Wo, bo):
    global _cached_nc
    c = FULL
    if _cached_nc is None:
        _cached_nc = build_nc(c)
    nc = _cached_nc

    in_maps = prep_in_maps(c, x, Wq, Wk, Wv, Wo, bo)
    res = run_bass_kernel_spmd(nc, in_maps, list(range(N_CORES)))
    out = np.concatenate([res.results[cid]["out"] for cid in range(N_CORES)], axis=0)
    return out.reshape(np.asarray(x).shape).astype(np.float32)


# revision 26
# speedup vs baseline: 1.0562x; 1.0437x over previous
"""Multi-head attention forward, distributed over 8 TRN2 NeuronCores.

Problem: x[2,2048,1024] -> QKV proj (16 heads x 64) -> softmax attention
-> output proj + bias -> [2,2048,1024], f32 I/O, bf16 tensor-engine compute.

Sharding: rows = flattened (batch, seq) = 4096 rows; core c owns rows
[c*512, (c+1)*512) -- cores 0-3 hold batch 0, cores 4-7 batch 1. Each core
projects Q/K/V for its own rows, all-gathers K^T and V (bf16) within its
4-core batch group in 4 key chunks, then computes attention for all 16
heads restricted to its 512 query rows, and the output projection.

Pipeline design (v3):
- A tiny dummy AllGather is emitted first so the NRT kernel-entry CC
  barrier (~20-40us rendezvous) runs while inputs are still streaming in,
  instead of serializing ahead of the first real gather.
- K^T is projected full-width, staged per 128-key chunk into a [256,1024]
  bounce (2KB rows; the v2 [*,128] layout produced 256B DMA descriptors
  that ran at ~6GB/s and stalled the whole projection pipeline).
- Each chunk's AllGather fires as soon as its K+V staging lands; attention
  runs chunk-by-chunk as gathers land. The ACT-engine exp stream (~147us
  for 16.8M score elements at 1 elem/lane/cycle) is the pacing resource;
  everything else hides behind it.
- Scores contract over K=64 via PE row-tiling: head 2p lives in partitions
  0-63 of the K^T/Q^T pair tiles, head 2p+1 in 64-127, and the two matmuls
  run concurrently in different PE row-groups (no zero-padded operands).
- V is augmented with a ones column so the attention matmul itself yields
  the softmax denominator in row 64 (exp has no max subtraction; scores
  are ~N(0,1) after the 1/sqrt(64) scale folded into the ACT scale).
- The V bounce->v_aug interleave copy runs on GpSimd so the Vector queue
  (PSUM evacuations, accumulator adds) never blocks chunk unpacking.
"""

import ml_dtypes
import numpy as np

import concourse.bass as bass
import concourse.mybir as mybir
import concourse.tile as tile
from concourse import bacc
from concourse.bass_utils import run_bass_kernel_spmd

BF = mybir.dt.bfloat16
F32 = mybir.dt.float32
F8 = mybir.dt.float8e4
P = 128

N_CORES = 8
GROUP = 4   # cores per batch group (one AllGather group)
NCH = 4     # AllGather chunks (pipelined)


class Cfg:
    def __init__(self, rpc, d, n_heads, head_dim):
        self.RPC = rpc            # query rows per core
        self.D = d                # model dim
        self.H = n_heads
        self.HD = head_dim
        assert n_heads * head_dim == d
        self.NT_D = d // P        # dim tiles (= head pairs)
        self.NT_R = rpc // P      # row tiles
        self.KEYS = rpc * GROUP   # keys per batch group
        self.NT_K = self.KEYS // P
        self.KPC = rpc // NCH     # local keys per chunk
        assert self.KPC == P      # one key tile per (rank, chunk)
        # bounce rows per rank chunk: K^T only (V is computed locally from
        # the host-replicated group x), [KPC, D] with row=partition of the
        # wide K^T tile, col=m*P+key (see ktw)
        self.CH_ROWS = self.KPC


FULL = Cfg(rpc=512, d=1024, n_heads=16, head_dim=64)


def _body(tc, nc, cfg, xT_in, xg_in, wq_in, wk_in, wv_in, wo_in, bo_in, out_ext):
    c = cfg
    AF = mybir.ActivationFunctionType
    rg = [list(range(GROUP)), list(range(GROUP, 2 * GROUP))]
    HD1 = c.HD + 1
    from contextlib import ExitStack

    stack = ExitStack()
    dram = stack.enter_context(tc.tile_pool(name="dram", bufs=1, space="DRAM"))
    const = stack.enter_context(tc.tile_pool(name="const", bufs=1))
    persist = stack.enter_context(tc.tile_pool(name="persist", bufs=1))

    kv_in = [dram.tile([c.CH_ROWS, c.D], BF, name=f"kv_in{h}") for h in range(NCH)]
    kv_g = [
        dram.tile([GROUP * c.CH_ROWS, c.D], BF, name=f"kv_g{h}") for h in range(NCH)
    ]
    ones_row = const.tile([1, P], BF, tag="ones_row", name="ones_row")
    nc.vector.memset(ones_row[:], 1.0)
    bo_sb = const.tile([1, c.D], BF, tag="bo", name="bo_sb")
    nc.sync.dma_start(bo_sb[:], bo_in[:, :])
    # Pre-warm the ACT exp table so the ~2.7us table load is off the
    # attention critical path.
    warm_act = const.tile([1, P], BF, tag="warm_act", name="warm_act")
    nc.scalar.activation(warm_act[:], ones_row[:], AF.Exp)

    def ptiles(shape, dt_, pfx, n, pool=None):
        pool = pool or persist
        return [pool.tile(shape, dt_, tag=f"{pfx}{t}", name=f"{pfx}{t}") for t in range(n)]

    xT = ptiles([P, c.RPC], BF, "xT", c.NT_D)
    xg = ptiles([P, c.KEYS], BF, "xg", c.NT_D)          # group x^T (all keys)
    qT = ptiles([P, c.RPC], BF, "qT", c.NT_D)
    attT = ptiles([P, c.RPC], BF, "attT", c.NT_D)
    kt = ptiles([P, c.KEYS], BF, "kt", c.NT_D)          # K^T pair tiles, all keys
    v_aug = ptiles([P, c.H * HD1], BF, "va", c.NT_K)    # V + ones col, per key tile
    acc_eo = ptiles([HD1, 2 * c.RPC], BF, "acc", c.NT_D)

    def emit_ag(h):
        nc.gpsimd.collective_compute(
            "AllGather",
            mybir.AluOpType.bypass,
            replica_groups=rg,
            ins=[kv_in[h][:].opt()],
            outs=[kv_g[h][:].opt()],
        )

    with (
        tc.tile_pool(name="stage", bufs=1) as stage,
        tc.tile_pool(name="wpool", bufs=1) as wpool,
        tc.tile_pool(name="proj_psum", bufs=3, space="PSUM") as proj_psum,
    ):
        wq_sb = ptiles([P, c.D], BF, "wq", c.NT_D, pool=wpool)
        wk_sb = ptiles([P, c.D], BF, "wk", c.NT_D, pool=wpool)
        # wide K^T staging tile: ktw[p, h*D + m*P + k] = K^T[m*P+p, h*KPC+k]
        ktw = stage.tile([P, NCH * c.D], BF, tag="ktw", name="ktw")

        # ---- phase 0: load x^T and weights (already bf16, pre-transposed).
        for t in range(c.NT_D):
            nc.sync.dma_start(xT[t][:], xT_in[t * P : (t + 1) * P, :])
            nc.sync.dma_start(wk_sb[t][:], wk_in[t * P : (t + 1) * P, :])
        for t in range(c.NT_D):
            nc.sync.dma_start(wq_sb[t][:], wq_in[t * P : (t + 1) * P, :])
        for h in range(NCH):
            for t in range(c.NT_D):
                nc.sync.dma_start(
                    xg[t][:, h * GROUP * P : (h + 1) * GROUP * P],
                    xg_in[t * P : (t + 1) * P, h * GROUP * P : (h + 1) * GROUP * P],
                )

        # ---- phase 1a: full-width K^T projection, evac sliced per chunk ----
        for m in range(c.NT_D):
            ps = proj_psum.tile([P, c.RPC], F32, tag="proj", name="kproj_ps")
            for k in range(c.NT_D):
                nc.tensor.matmul(
                    ps[:],
                    wk_sb[k][:, m * P : (m + 1) * P],
                    xT[k][:],
                    start=(k == 0),
                    stop=(k == c.NT_D - 1),
                )
            # one strided copy scatters the 4 chunk slices of this m
            nc.vector.tensor_copy(
                ktw[:].rearrange("p (h mk) -> p h mk", h=NCH)[
                    :, :, m * P : (m + 1) * P
                ],
                ps[:].rearrange("p (h k) -> p h k", h=NCH),
            )

        # ---- phase 1b: stage each K chunk -> AllGather (K only) ----
        for h in range(NCH):
            nc.sync.dma_start(
                kv_in[h][0 : c.KPC, :], ktw[:, h * c.D : (h + 1) * c.D]
            )
            emit_ag(h)
            if h == 0:
                # Q^T projection overlaps the first AllGather's flight time
                for m in range(c.NT_D):
                    ps = proj_psum.tile([P, c.RPC], F32, tag="proj", name="qproj_ps")
                    for k in range(c.NT_D):
                        nc.tensor.matmul(
                            ps[:],
                            wq_sb[k][:, m * P : (m + 1) * P],
                            xT[k][:],
                            start=(k == 0),
                            stop=(k == c.NT_D - 1),
                        )
                    nc.vector.tensor_copy(qT[m][:], ps[:])

    # ---- phases 2+3: per chunk: unpack gathered K/V, then attention ----
    # key-tile order is chunk-major: tile j = h*GROUP + r covers gathered
    # chunk h of rank r; kt cols [j*P, (j+1)*P) and v_aug[j] use it.
    with (
        tc.tile_pool(name="wopool", bufs=1) as wopool,
        tc.tile_pool(name="pT", bufs=4) as pT_pool,
        tc.tile_pool(name="small", bufs=4) as small,
        tc.tile_pool(name="sc_psum", bufs=2, space="PSUM") as sc_psum,
        tc.tile_pool(name="att_psum", bufs=1, space="PSUM") as att_psum,
        tc.tile_pool(name="vproj_psum", bufs=2, space="PSUM") as vproj_psum,
    ):
        wo_sb = ptiles([P, c.D], BF, "wo", c.NT_D, pool=wopool)
        wv_sb = ptiles([P, c.D], BF, "wv", c.NT_D, pool=wopool)
        for t in range(c.NT_D):
            nc.sync.dma_start(wv_sb[t][:], wv_in[t * P : (t + 1) * P, :])

        def v_half(j, n):
            """One 512-col half of V-tile j's projection (see v_tile)."""
            gb = (j % GROUP) * c.RPC + (j // GROUP) * c.KPC
            ps = vproj_psum.tile([P, c.RPC], F32, tag="vproj", name="vproj_ps")
            for k in range(c.NT_D):
                nc.tensor.matmul(
                    ps[:],
                    xg[k][:, gb : gb + P],
                    wv_sb[k][:, n * c.RPC : (n + 1) * c.RPC],
                    start=(k == 0),
                    stop=(k == c.NT_D - 1),
                )
            nc.vector.tensor_copy(
                v_aug[j][:, n * 8 * HD1 : (n + 1) * 8 * HD1].rearrange(
                    "p (x e) -> p x e", e=HD1
                )[:, :, 0 : c.HD],
                ps[:].rearrange("p (x e) -> p x e", e=c.HD),
            )
            if n == 1:
                ones_col = v_aug[j][:].rearrange("p (x e) -> p x e", e=HD1)[
                    :, :, c.HD : HD1
                ]
                nc.gpsimd.memset(ones_col, 1.0)

        def v_tile(j):
            """V projection for key tile j from the replicated group x,
            evacuated straight into the ones-augmented [head, 65] layout
            (one strided copy per half).  Key tile j follows the gathered
            chunk-major order: j = chunk*GROUP + rank covers group rows
            rank*RPC + chunk*KPC (xg columns are in natural group order)."""
            gb = (j % GROUP) * c.RPC + (j // GROUP) * c.KPC
            for n in range(2):
                ps = vproj_psum.tile([P, c.RPC], F32, tag="vproj", name="vproj_ps")
                for k in range(c.NT_D):
                    nc.tensor.matmul(
                        ps[:],
                        xg[k][:, gb : gb + P],
                        wv_sb[k][:, n * c.RPC : (n + 1) * c.RPC],
                        start=(k == 0),
                        stop=(k == c.NT_D - 1),
                    )
                nc.vector.tensor_copy(
                    v_aug[j][:, n * 8 * HD1 : (n + 1) * 8 * HD1].rearrange(
                        "p (x e) -> p x e", e=HD1
                    )[:, :, 0 : c.HD],
                    ps[:].rearrange("p (x e) -> p x e", e=c.HD),
                )
            ones_col = v_aug[j][:].rearrange("p (x e) -> p x e", e=HD1)[
                :, :, c.HD : HD1
            ]
            nc.gpsimd.memset(ones_col, 1.0)

        # chunk-0 V tiles up front (TensorE work, overlaps the first gather)
        for j in range(GROUP):
            v_tile(j)

        for h in range(NCH):
            if h == 1:
                # Wo loads are off the critical path; emit after chunk 0's
                # unpack so they don't contend with the staging window
                for t in range(c.NT_D):
                    nc.sync.dma_start(wo_sb[t][:], wo_in[t * P : (t + 1) * P, :])

            # unpack K: per m, one DMA grabs all 4 ranks' key blocks
            for m in range(c.NT_D):
                nc.sync.dma_start(
                    kt[m][:, h * GROUP * P : (h + 1) * GROUP * P].rearrange(
                        "p (r k) -> p r k", r=GROUP
                    ),
                    kv_g[h][:, m * P : (m + 1) * P].rearrange(
                        "(r q) k -> q r k", r=GROUP
                    )[0:P, :, :],
                )
            for p in range(c.NT_D):
                he, ho = 2 * p, 2 * p + 1
                att_eo = att_psum.tile([HD1, 2 * c.RPC], F32, tag="att_eo", name="att_eo")
                for i in range(GROUP):
                    j = h * GROUP + i
                    col = j * P
                    sc = sc_psum.tile([P, 2 * c.RPC], F32, tag="scores", name="sc_ps")
                    # even/odd heads of the pair run concurrently in PE
                    # row-groups 0 and 2 (K=64 row tiling)
                    nc.tensor.matmul(
                        sc[:, 0 : c.RPC],
                        kt[p][0 : c.HD, col : col + P],
                        qT[p][0 : c.HD, :],
                        start=True,
                        stop=True,
                    )
                    nc.tensor.matmul(
                        sc[:, c.RPC : 2 * c.RPC],
                        kt[p][c.HD : P, col : col + P],
                        qT[p][c.HD : P, :],
                        start=True,
                        stop=True,
                    )
                    pT = pT_pool.tile([P, 2 * c.RPC], BF, tag="pT", name="pT")
                    nc.scalar.activation(
                        pT[:], sc[:], AF.Exp, scale=1.0 / float(np.sqrt(c.HD))
                    )
                    nc.tensor.matmul(
                        att_eo[:, 0 : c.RPC],
                        v_aug[j][:, he * HD1 : (he + 1) * HD1],
                        pT[:, 0 : c.RPC],
                        start=(i == 0),
                        stop=(i == GROUP - 1),
                    )
                    nc.tensor.matmul(
                        att_eo[:, c.RPC : 2 * c.RPC],
                        v_aug[j][:, ho * HD1 : (ho + 1) * HD1],
                        pT[:, c.RPC : 2 * c.RPC],
                        start=(i == 0),
                        stop=(i == GROUP - 1),
                    )
                if h == 0:
                    nc.vector.tensor_copy(acc_eo[p][:], att_eo[:])
                else:
                    nc.vector.tensor_add(acc_eo[p][:], att_eo[:], acc_eo[p][:])

                if h + 1 < NCH:
                    # next chunk's V projections, one half-tile chain per
                    # pair slot: ~2us of TensorE work per slot stays under
                    # the exp stream's ~4.6us pair cadence
                    v_half((h + 1) * GROUP + p // 2, p % 2)

                if h == NCH - 1:
                    # normalization: denominators live in row HD of the accs.
                    # reciprocal_approx_fast is a custom DVE program -- feed
                    # it a partition-0-based tile, not a row-64 slice.
                    den = small.tile([1, 2 * c.RPC], F32, tag="den", name="den", bufs=1)
                    nc.vector.tensor_copy(den[:], acc_eo[p][c.HD : HD1, :])
                    rcp = small.tile([1, 2 * c.RPC], F32, tag="rcp", name="rcp", bufs=1)
                    nc.vector.reciprocal_approx_fast(rcp[:], den[:])
                    rcpb = small.tile([c.HD, 2 * c.RPC], F32, tag="rcpb", name="rcpb", bufs=1)
                    nc.gpsimd.partition_broadcast(rcpb[:], rcp[:])
                    nc.vector.tensor_mul(
                        attT[p][0 : c.HD, :], acc_eo[p][0 : c.HD, 0 : c.RPC],
                        rcpb[:, 0 : c.RPC],
                    )
                    nc.vector.tensor_mul(
                        attT[p][c.HD : P, :], acc_eo[p][0 : c.HD, c.RPC : 2 * c.RPC],
                        rcpb[:, c.RPC : 2 * c.RPC],
                    )

        # ---- phase 4: output projection + bias ----
        for rt in range(c.NT_R):
            out_sb = small.tile([P, c.D], F32, tag="outsb", name="outsb", bufs=2)
            for n in range(2):
                po = sc_psum.tile([P, c.RPC], F32, tag="scores", name="out_ps")
                for k in range(c.NT_D):
                    nc.tensor.matmul(
                        po[:],
                        attT[k][:, rt * P : (rt + 1) * P],
                        wo_sb[k][:, n * c.RPC : (n + 1) * c.RPC],
                        start=(k == 0),
                        stop=False,
                    )
                nc.tensor.matmul(
                    po[:],
                    ones_row[:],
                    bo_sb[:, n * c.RPC : (n + 1) * c.RPC],
                    start=False,
                    stop=True,
                )
                nc.vector.tensor_copy(out_sb[:, n * c.RPC : (n + 1) * c.RPC], po[:])
            nc.sync.dma_start(out_ext[rt * P : (rt + 1) * P, :], out_sb[:])

    stack.close()


def build_nc(cfg):
    nc = bacc.Bacc(
        "TRN2", target_bir_lowering=False, debug=False, num_devices=N_CORES
    )
    c = cfg
    xT_in = nc.dram_tensor("xT", [c.D, c.RPC], BF, kind="ExternalInput")
    xg_in = nc.dram_tensor("xg", [c.D, c.KEYS], BF, kind="ExternalInput")
    wq_in = nc.dram_tensor("Wq", [c.D, c.D], BF, kind="ExternalInput")
    wk_in = nc.dram_tensor("Wk", [c.D, c.D], BF, kind="ExternalInput")
    wv_in = nc.dram_tensor("Wv", [c.D, c.D], BF, kind="ExternalInput")
    wo_in = nc.dram_tensor("Wo", [c.D, c.D], BF, kind="ExternalInput")
    bo_in = nc.dram_tensor("bo", [1, c.D], BF, kind="ExternalInput")
    out_ext = nc.dram_tensor("out", [c.RPC, c.D], F32, kind="ExternalOutput")

    with tile.TileContext(nc) as tc:
        _body(
            tc, nc, cfg,
            xT_in.ap(), xg_in.ap(), wq_in.ap(), wk_in.ap(), wv_in.ap(),
            wo_in.ap(), bo_in.ap(), out_ext.ap(),
        )
    nc.compile()
    return nc


_cached_nc = None


def _bf16(a):
    return np.ascontiguousarray(np.asarray(a, dtype=np.float32)).astype(
        ml_dtypes.bfloat16
    )


def prep_in_maps(c, x, Wq, Wk, Wv, Wo, bo):
    xf = np.ascontiguousarray(np.asarray(x, dtype=np.float32)).reshape(-1, c.D)
    wq, wk, wv, wo = _bf16(Wq), _bf16(Wk), _bf16(Wv), _bf16(Wo)
    bob = _bf16(bo).reshape(1, c.D)
    xgs = [
        np.ascontiguousarray(
            xf[g * c.KEYS : (g + 1) * c.KEYS].T.astype(ml_dtypes.bfloat16)
        )
        for g in range(N_CORES // GROUP)
    ]
    return [
        {
            "xT": np.ascontiguousarray(
                xf[cid * c.RPC : (cid + 1) * c.RPC].T.astype(ml_dtypes.bfloat16)
            ),
            "xg": xgs[cid // GROUP],
            "Wq": wq, "Wk": wk, "Wv": wv, "Wo": wo, "bo": bob,
        }
        for cid in range(N_CORES)
    ]


def kernel(x, Wq, Wk, Wv, Wo, bo):
    global _cached_nc
    c = FULL
    if _cached_nc is None:
        _cached_nc = build_nc(c)
    nc = _cached_nc

    in_maps = prep_in_maps(c, x, Wq, Wk, Wv, Wo, bo)
    res = run_bass_kernel_spmd(nc, in_maps, list(range(N_CORES)))
    out = np.concatenate([res.results[cid]["out"] for cid in range(N_CORES)], axis=0)
    return out.reshape(np.asarray(x).shape).astype(np.float32)


# revision 27
# speedup vs baseline: 1.0626x; 1.0061x over previous
"""Multi-head attention forward, distributed over 8 TRN2 NeuronCores.

Problem: x[2,2048,1024] -> QKV proj (16 heads x 64) -> softmax attention
-> output proj + bias -> [2,2048,1024], f32 I/O, bf16 tensor-engine compute.

Sharding: rows = flattened (batch, seq) = 4096 rows; core c owns rows
[c*512, (c+1)*512) -- cores 0-3 hold batch 0, cores 4-7 batch 1. Each core
projects Q/K/V for its own rows, all-gathers K^T and V (bf16) within its
4-core batch group in 4 key chunks, then computes attention for all 16
heads restricted to its 512 query rows, and the output projection.

Pipeline design (v3):
- A tiny dummy AllGather is emitted first so the NRT kernel-entry CC
  barrier (~20-40us rendezvous) runs while inputs are still streaming in,
  instead of serializing ahead of the first real gather.
- K^T is projected full-width, staged per 128-key chunk into a [256,1024]
  bounce (2KB rows; the v2 [*,128] layout produced 256B DMA descriptors
  that ran at ~6GB/s and stalled the whole projection pipeline).
- Each chunk's AllGather fires as soon as its K+V staging lands; attention
  runs chunk-by-chunk as gathers land. The ACT-engine exp stream (~147us
  for 16.8M score elements at 1 elem/lane/cycle) is the pacing resource;
  everything else hides behind it.
- Scores contract over K=64 via PE row-tiling: head 2p lives in partitions
  0-63 of the K^T/Q^T pair tiles, head 2p+1 in 64-127, and the two matmuls
  run concurrently in different PE row-groups (no zero-padded operands).
- V is augmented with a ones column so the attention matmul itself yields
  the softmax denominator in row 64 (exp has no max subtraction; scores
  are ~N(0,1) after the 1/sqrt(64) scale folded into the ACT scale).
- The V bounce->v_aug interleave copy runs on GpSimd so the Vector queue
  (PSUM evacuations, accumulator adds) never blocks chunk unpacking.
"""

import ml_dtypes
import numpy as np

import concourse.bass as bass
import concourse.mybir as mybir
import concourse.tile as tile
from concourse import bacc
from concourse.bass_utils import run_bass_kernel_spmd

BF = mybir.dt.bfloat16
F32 = mybir.dt.float32
F8 = mybir.dt.float8e4
P = 128

N_CORES = 8
GROUP = 4   # cores per batch group (one AllGather group)
NCH = 4     # AllGather chunks (pipelined)


class Cfg:
    def __init__(self, rpc, d, n_heads, head_dim):
        self.RPC = rpc            # query rows per core
        self.D = d                # model dim
        self.H = n_heads
        self.HD = head_dim
        assert n_heads * head_dim == d
        self.NT_D = d // P        # dim tiles (= head pairs)
        self.NT_R = rpc // P      # row tiles
        self.KEYS = rpc * GROUP   # keys per batch group
        self.NT_K = self.KEYS // P
        self.KPC = rpc // NCH     # local keys per chunk
        assert self.KPC == P      # one key tile per (rank, chunk)
        # bounce rows per rank chunk: K^T only (V is computed locally from
        # the host-replicated group x), [KPC, D] with row=partition of the
        # wide K^T tile, col=m*P+key (see ktw)
        self.CH_ROWS = self.KPC


FULL = Cfg(rpc=512, d=1024, n_heads=16, head_dim=64)


def _body(tc, nc, cfg, xT_in, xg_in, wq_in, wk_in, wv_in, wo_in, bo_in, out_ext):
    c = cfg
    AF = mybir.ActivationFunctionType
    rg = [list(range(GROUP)), list(range(GROUP, 2 * GROUP))]
    HD1 = c.HD + 1
    from contextlib import ExitStack

    stack = ExitStack()
    dram = stack.enter_context(tc.tile_pool(name="dram", bufs=1, space="DRAM"))
    const = stack.enter_context(tc.tile_pool(name="const", bufs=1))
    persist = stack.enter_context(tc.tile_pool(name="persist", bufs=1))

    kv_in = [dram.tile([c.CH_ROWS, c.D], BF, name=f"kv_in{h}") for h in range(NCH)]
    kv_g = [
        dram.tile([GROUP * c.CH_ROWS, c.D], BF, name=f"kv_g{h}") for h in range(NCH)
    ]
    ones_row = const.tile([1, P], BF, tag="ones_row", name="ones_row")
    nc.vector.memset(ones_row[:], 1.0)
    bo_sb = const.tile([1, c.D], BF, tag="bo", name="bo_sb")
    nc.sync.dma_start(bo_sb[:], bo_in[:, :])
    # Pre-warm the ACT exp table so the ~2.7us table load is off the
    # attention critical path.
    warm_act = const.tile([1, P], BF, tag="warm_act", name="warm_act")
    nc.scalar.activation(warm_act[:], ones_row[:], AF.Exp)

    def ptiles(shape, dt_, pfx, n, pool=None):
        pool = pool or persist
        return [pool.tile(shape, dt_, tag=f"{pfx}{t}", name=f"{pfx}{t}") for t in range(n)]

    xT = ptiles([P, c.RPC], BF, "xT", c.NT_D)
    xg = ptiles([P, c.KEYS], BF, "xg", c.NT_D)          # group x^T (all keys)
    qT = ptiles([P, c.RPC], BF, "qT", c.NT_D)
    attT = ptiles([P, c.RPC], BF, "attT", c.NT_D)
    kt = ptiles([P, c.KEYS], BF, "kt", c.NT_D)          # K^T pair tiles, all keys
    v_aug = ptiles([P, c.H * HD1], BF, "va", c.NT_K)    # V + ones col, per key tile
    acc_eo = ptiles([HD1, 2 * c.RPC], BF, "acc", c.NT_D)

    def emit_ag(h):
        nc.gpsimd.collective_compute(
            "AllGather",
            mybir.AluOpType.bypass,
            replica_groups=rg,
            ins=[kv_in[h][:].opt()],
            outs=[kv_g[h][:].opt()],
        )

    with (
        tc.tile_pool(name="stage", bufs=1) as stage,
        tc.tile_pool(name="wpool", bufs=1) as wpool,
        tc.tile_pool(name="proj_psum", bufs=3, space="PSUM") as proj_psum,
    ):
        wq_sb = ptiles([P, c.D], BF, "wq", c.NT_D, pool=wpool)
        wk_sb = ptiles([P, c.D], BF, "wk", c.NT_D, pool=wpool)
        # wide K^T staging tile: ktw[p, h*D + m*P + k] = K^T[m*P+p, h*KPC+k]
        ktw = stage.tile([P, NCH * c.D], BF, tag="ktw", name="ktw")

        # ---- phase 0: load x^T and weights (already bf16, pre-transposed).
        for t in range(c.NT_D):
            nc.sync.dma_start(xT[t][:], xT_in[t * P : (t + 1) * P, :])
            nc.sync.dma_start(wk_sb[t][:], wk_in[t * P : (t + 1) * P, :])
        for t in range(c.NT_D):
            nc.sync.dma_start(wq_sb[t][:], wq_in[t * P : (t + 1) * P, :])
        for h in range(NCH):
            for t in range(c.NT_D):
                nc.sync.dma_start(
                    xg[t][:, h * GROUP * P : (h + 1) * GROUP * P],
                    xg_in[t * P : (t + 1) * P, h * GROUP * P : (h + 1) * GROUP * P],
                )

        # ---- phase 1a: full-width K^T projection, evac sliced per chunk ----
        for m in range(c.NT_D):
            ps = proj_psum.tile([P, c.RPC], F32, tag="proj", name="kproj_ps")
            for k in range(c.NT_D):
                nc.tensor.matmul(
                    ps[:],
                    wk_sb[k][:, m * P : (m + 1) * P],
                    xT[k][:],
                    start=(k == 0),
                    stop=(k == c.NT_D - 1),
                )
            # one strided copy scatters the 4 chunk slices of this m
            nc.vector.tensor_copy(
                ktw[:].rearrange("p (h mk) -> p h mk", h=NCH)[
                    :, :, m * P : (m + 1) * P
                ],
                ps[:].rearrange("p (h k) -> p h k", h=NCH),
            )

        # ---- phase 1b: stage each K chunk -> AllGather (K only) ----
        for h in range(NCH):
            nc.sync.dma_start(
                kv_in[h][0 : c.KPC, :], ktw[:, h * c.D : (h + 1) * c.D]
            )
            emit_ag(h)
            if h == 0:
                # Q^T projection overlaps the first AllGather's flight time
                for m in range(c.NT_D):
                    ps = proj_psum.tile([P, c.RPC], F32, tag="proj", name="qproj_ps")
                    for k in range(c.NT_D):
                        nc.tensor.matmul(
                            ps[:],
                            wq_sb[k][:, m * P : (m + 1) * P],
                            xT[k][:],
                            start=(k == 0),
                            stop=(k == c.NT_D - 1),
                        )
                    nc.vector.tensor_copy(qT[m][:], ps[:])

    # ---- phases 2+3: per chunk: unpack gathered K/V, then attention ----
    # key-tile order is chunk-major: tile j = h*GROUP + r covers gathered
    # chunk h of rank r; kt cols [j*P, (j+1)*P) and v_aug[j] use it.
    with (
        tc.tile_pool(name="wopool", bufs=1) as wopool,
        tc.tile_pool(name="pT", bufs=4) as pT_pool,
        tc.tile_pool(name="small", bufs=4) as small,
        tc.tile_pool(name="sc_psum", bufs=2, space="PSUM") as sc_psum,
        tc.tile_pool(name="att_psum", bufs=1, space="PSUM") as att_psum,
        tc.tile_pool(name="vproj_psum", bufs=2, space="PSUM") as vproj_psum,
    ):
        wo_sb = ptiles([P, c.D], BF, "wo", c.NT_D, pool=wopool)
        wv_sb = ptiles([P, c.D], BF, "wv", c.NT_D, pool=wopool)
        for t in range(c.NT_D):
            nc.sync.dma_start(wv_sb[t][:], wv_in[t * P : (t + 1) * P, :])

        def v_half(j, n):
            """One 512-col half of V-tile j's projection (see v_tile)."""
            gb = (j % GROUP) * c.RPC + (j // GROUP) * c.KPC
            ps = vproj_psum.tile([P, c.RPC], F32, tag="vproj", name="vproj_ps")
            for k in range(c.NT_D):
                nc.tensor.matmul(
                    ps[:],
                    xg[k][:, gb : gb + P],
                    wv_sb[k][:, n * c.RPC : (n + 1) * c.RPC],
                    start=(k == 0),
                    stop=(k == c.NT_D - 1),
                )
            nc.vector.tensor_copy(
                v_aug[j][:, n * 8 * HD1 : (n + 1) * 8 * HD1].rearrange(
                    "p (x e) -> p x e", e=HD1
                )[:, :, 0 : c.HD],
                ps[:].rearrange("p (x e) -> p x e", e=c.HD),
            )
            if n == 1:
                ones_col = v_aug[j][:].rearrange("p (x e) -> p x e", e=HD1)[
                    :, :, c.HD : HD1
                ]
                nc.gpsimd.memset(ones_col, 1.0)

        def v_tile(j):
            """V projection for key tile j from the replicated group x,
            evacuated straight into the ones-augmented [head, 65] layout
            (one strided copy per half).  Key tile j follows the gathered
            chunk-major order: j = chunk*GROUP + rank covers group rows
            rank*RPC + chunk*KPC (xg columns are in natural group order)."""
            gb = (j % GROUP) * c.RPC + (j // GROUP) * c.KPC
            for n in range(2):
                ps = vproj_psum.tile([P, c.RPC], F32, tag="vproj", name="vproj_ps")
                for k in range(c.NT_D):
                    nc.tensor.matmul(
                        ps[:],
                        xg[k][:, gb : gb + P],
                        wv_sb[k][:, n * c.RPC : (n + 1) * c.RPC],
                        start=(k == 0),
                        stop=(k == c.NT_D - 1),
                    )
                nc.vector.tensor_copy(
                    v_aug[j][:, n * 8 * HD1 : (n + 1) * 8 * HD1].rearrange(
                        "p (x e) -> p x e", e=HD1
                    )[:, :, 0 : c.HD],
                    ps[:].rearrange("p (x e) -> p x e", e=c.HD),
                )
            ones_col = v_aug[j][:].rearrange("p (x e) -> p x e", e=HD1)[
                :, :, c.HD : HD1
            ]
            nc.gpsimd.memset(ones_col, 1.0)

        # ALL V tiles up front: ~56us of contiguous TensorE work fills the
        # barrier+AG0 wait window, so the PE enters the attention phase at
        # HAM K=8/8 instead of oscillating cold (434ns N=512 matmuls) the
        # whole way through
        for j in range(c.NT_K):
            v_tile(j)

        for h in range(NCH):
            if h == 1:
                # Wo loads are off the critical path; emit after chunk 0's
                # unpack so they don't contend with the staging window
                for t in range(c.NT_D):
                    nc.sync.dma_start(wo_sb[t][:], wo_in[t * P : (t + 1) * P, :])

            # unpack K: per m, one DMA grabs all 4 ranks' key blocks
            for m in range(c.NT_D):
                nc.sync.dma_start(
                    kt[m][:, h * GROUP * P : (h + 1) * GROUP * P].rearrange(
                        "p (r k) -> p r k", r=GROUP
                    ),
                    kv_g[h][:, m * P : (m + 1) * P].rearrange(
                        "(r q) k -> q r k", r=GROUP
                    )[0:P, :, :],
                )
            for p in range(c.NT_D):
                he, ho = 2 * p, 2 * p + 1
                att_eo = att_psum.tile([HD1, 2 * c.RPC], F32, tag="att_eo", name="att_eo")
                for i in range(GROUP):
                    j = h * GROUP + i
                    col = j * P
                    sc = sc_psum.tile([P, 2 * c.RPC], F32, tag="scores", name="sc_ps")
                    # even/odd heads of the pair run concurrently in PE
                    # row-groups 0 and 2 (K=64 row tiling)
                    nc.tensor.matmul(
                        sc[:, 0 : c.RPC],
                        kt[p][0 : c.HD, col : col + P],
                        qT[p][0 : c.HD, :],
                        start=True,
                        stop=True,
                    )
                    nc.tensor.matmul(
                        sc[:, c.RPC : 2 * c.RPC],
                        kt[p][c.HD : P, col : col + P],
                        qT[p][c.HD : P, :],
                        start=True,
                        stop=True,
                    )
                    pT = pT_pool.tile([P, 2 * c.RPC], BF, tag="pT", name="pT")
                    nc.scalar.activation(
                        pT[:], sc[:], AF.Exp, scale=1.0 / float(np.sqrt(c.HD))
                    )
                    nc.tensor.matmul(
                        att_eo[:, 0 : c.RPC],
                        v_aug[j][:, he * HD1 : (he + 1) * HD1],
                        pT[:, 0 : c.RPC],
                        start=(i == 0),
                        stop=(i == GROUP - 1),
                    )
                    nc.tensor.matmul(
                        att_eo[:, c.RPC : 2 * c.RPC],
                        v_aug[j][:, ho * HD1 : (ho + 1) * HD1],
                        pT[:, c.RPC : 2 * c.RPC],
                        start=(i == 0),
                        stop=(i == GROUP - 1),
                    )
                if h == 0:
                    nc.vector.tensor_copy(acc_eo[p][:], att_eo[:])
                else:
                    nc.vector.tensor_add(acc_eo[p][:], att_eo[:], acc_eo[p][:])

                if h == NCH - 1:
                    # normalization: denominators live in row HD of the accs.
                    # reciprocal_approx_fast is a custom DVE program -- feed
                    # it a partition-0-based tile, not a row-64 slice.
                    den = small.tile([1, 2 * c.RPC], F32, tag="den", name="den", bufs=1)
                    nc.vector.tensor_copy(den[:], acc_eo[p][c.HD : HD1, :])
                    rcp = small.tile([1, 2 * c.RPC], F32, tag="rcp", name="rcp", bufs=1)
                    nc.vector.reciprocal_approx_fast(rcp[:], den[:])
                    rcpb = small.tile([c.HD, 2 * c.RPC], F32, tag="rcpb", name="rcpb", bufs=1)
                    nc.gpsimd.partition_broadcast(rcpb[:], rcp[:])
                    nc.vector.tensor_mul(
                        attT[p][0 : c.HD, :], acc_eo[p][0 : c.HD, 0 : c.RPC],
                        rcpb[:, 0 : c.RPC],
                    )
                    nc.vector.tensor_mul(
                        attT[p][c.HD : P, :], acc_eo[p][0 : c.HD, c.RPC : 2 * c.RPC],
                        rcpb[:, c.RPC : 2 * c.RPC],
                    )

        # ---- phase 4: output projection + bias ----
        for rt in range(c.NT_R):
            out_sb = small.tile([P, c.D], F32, tag="outsb", name="outsb", bufs=2)
            for n in range(2):
                po = sc_psum.tile([P, c.RPC], F32, tag="scores", name="out_ps")
                for k in range(c.NT_D):
                    nc.tensor.matmul(
                        po[:],
                        attT[k][:, rt * P : (rt + 1) * P],
                        wo_sb[k][:, n * c.RPC : (n + 1) * c.RPC],
                        start=(k == 0),
                        stop=False,
                    )
                nc.tensor.matmul(
                    po[:],
                    ones_row[:],
                    bo_sb[:, n * c.RPC : (n + 1) * c.RPC],
                    start=False,
                    stop=True,
                )
                nc.vector.tensor_copy(out_sb[:, n * c.RPC : (n + 1) * c.RPC], po[:])
            nc.sync.dma_start(out_ext[rt * P : (rt + 1) * P, :], out_sb[:])

    stack.close()


def build_nc(cfg):
    nc = bacc.Bacc(
        "TRN2", target_bir_lowering=False, debug=False, num_devices=N_CORES
    )
    c = cfg
    xT_in = nc.dram_tensor("xT", [c.D, c.RPC], BF, kind="ExternalInput")
    xg_in = nc.dram_tensor("xg", [c.D, c.KEYS], BF, kind="ExternalInput")
    wq_in = nc.dram_tensor("Wq", [c.D, c.D], BF, kind="ExternalInput")
    wk_in = nc.dram_tensor("Wk", [c.D, c.D], BF, kind="ExternalInput")
    wv_in = nc.dram_tensor("Wv", [c.D, c.D], BF, kind="ExternalInput")
    wo_in = nc.dram_tensor("Wo", [c.D, c.D], BF, kind="ExternalInput")
    bo_in = nc.dram_tensor("bo", [1, c.D], BF, kind="ExternalInput")
    out_ext = nc.dram_tensor("out", [c.RPC, c.D], F32, kind="ExternalOutput")

    with tile.TileContext(nc) as tc:
        _body(
            tc, nc, cfg,
            xT_in.ap(), xg_in.ap(), wq_in.ap(), wk_in.ap(), wv_in.ap(),
            wo_in.ap(), bo_in.ap(), out_ext.ap(),
        )
    nc.compile()
    return nc


_cached_nc = None


def _bf16(a):
    return np.ascontiguousarray(np.asarray(a, dtype=np.float32)).astype(
        ml_dtypes.bfloat16
    )


def prep_in_maps(c, x, Wq, Wk, Wv, Wo, bo):
    xf = np.ascontiguousarray(np.asarray(x, dtype=np.float32)).reshape(-1, c.D)
    wq, wk, wv, wo = _bf16(Wq), _bf16(Wk), _bf16(Wv), _bf16(Wo)
    bob = _bf16(bo).reshape(1, c.D)
    xgs = [
        np.ascontiguousarray(
            xf[g * c.KEYS : (g + 1) * c.KEYS].T.astype(ml_dtypes.bfloat16)
        )
        for g in range(N_CORES // GROUP)
    ]
    return [
        {
            "xT": np.ascontiguousarray(
                xf[cid * c.RPC : (cid + 1) * c.RPC].T.astype(ml_dtypes.bfloat16)
            ),
            "xg": xgs[cid // GROUP],
            "Wq": wq, "Wk": wk, "Wv": wv, "Wo": wo, "bo": bob,
        }
        for cid in range(N_CORES)
    ]


def kernel(x, Wq, Wk, Wv, Wo, bo):
    global _cached_nc
    c = FULL
    if _cached_nc is None:
        _cached_nc = build_nc(c)
    nc = _cached_nc

    in_maps = prep_in_maps(c, x, Wq, Wk, Wv, Wo, bo)
    res = run_bass_kernel_spmd(nc, in_maps, list(range(N_CORES)))
    out = np.concatenate([res.results[cid]["out"] for cid in range(N_CORES)], axis=0)
    return out.reshape(np.asarray(x).shape).astype(np.float32)
